# revision 6
# baseline (speedup 1.0000x reference)
"""Single-head causal attention with RoPE + padding mask, data-parallel
over batch across 8 TRN2 NeuronCores (one batch element per core).

Per core (T=4096, C=128, HS=64):
  q = rope(x @ Wq); k = rope(x @ Wk); v = x @ Wv
  S^T[j,i] = k[j]·q[i]           (scores, transposed layout: partition=j)
  P^T = exp(S^T/sqrt(C) + maskbias[j]) * tri(i>=j)   (no max-subtraction:
        scores are O(0.1) for this problem so exp is numerically safe)
  outT[d,i] = sum_j v[j,d] P^T[j,i];  rowsum via a ones-column in v
  out[i,d] = outT[d,i] / rowsum[i]

Layout tricks (all host-side prep is pure layout/precision prep):
  - x is passed pre-transposed per-core as xT [C,T] bf16.
  - RoPE pair-swap folded into extra weight matrices Wq_swap/Wk_swap
    (swap adjacent columns), so rope = qraw*cos2 + qswap*sin2s with
    cos2/sin2s passed pre-expanded [HS,T] from host.
  - The padding mask enters as a 65th contraction row of the S^T matmul
    (k-plus row = 0/-1e30, q-plus row = 1), so exp needs no per-tile bias.
"""

import numpy as np

T, C, HS = 4096, 128, 64
N_CORES = 8
NT = T // 128      # 32 j-tiles of 128
NCH = T // 512     # 8 i-chunks of 512
JGRP = 2           # j-tiles per exp group (PSUM-bank budget bound)
SCALE = float(1.0 / np.sqrt(np.float32(C)))
NEG = -1e30

_CACHE = {}


def _install_tile_drain_patch(tile_mod):
    """This container's walrus rejects instructions with >2 sem waits; split
    Tile's final global drain into one drain per ticked processor."""
    import bass_rust
    from concourse.vector_clock import ScopedClock

    def _patched(self, tick_clock, wait_clock):
        gc = tick_clock.global_clock
        for i in range(len(gc)):
            if gc[i] <= 0:
                continue
            v = bass_rust.VectorClock()
            v.require_at_least(i, gc[i])
            d = self.nc.sync.drain()
            wait_clock.add_sem_waits(d.ins, ScopedClock({None: v}))
        self.nc.all_engine_barrier()
        assert self.sems is not None
        popped = self.nc._tile_sem_poison_stack.pop()
        assert popped is self._sem_poison
        self.nc.clear_and_free_semaphores(list(self.sems.allocated().values()))
        self.nc.all_engine_barrier()

    tile_mod.TileContext._drain_and_barrier = _patched


def _split_excess_waits(nc, mybir, limit=1):
    """This container's walrus rejects instructions with >limit sem waits.
    Hoist excess waits onto standalone EventSemaphore instructions inserted
    just before the offending instruction on the same engine queue."""
    ctr = 0
    for f in nc.m.functions:
        for b in f.blocks:
            il = b.instructions
            out = []
            changed = False
            for ins in il:
                si = ins.sync_info
                waits = list(si.on_wait) if si and si.on_wait else []
                if len(waits) > limit:
                    changed = True
                    excess = waits[: len(waits) - limit]
                    keep = waits[len(waits) - limit :]
                    for i in range(0, len(excess), limit):
                        chunk = excess[i : i + limit]
                        ev = mybir.InstEventSemaphore(
                            name=f"I-waitsplit-{ctr}",
                            engine=ins.engine,
                            ins=[],
                            outs=[],
                            sync_info=mybir.SyncInfo(on_wait=chunk, on_update=[]),
                        )
                        ctr += 1
                        nc.register_instruction(ev)
                        out.append(ev)
                    si.on_wait = keep
                out.append(ins)
            if changed:
                b.instructions = out


def _build_nc():
    import concourse.bass as bass
    import concourse.mybir as mybir
    from concourse import tile, masks

    _install_tile_drain_patch(tile)

    DT = mybir.dt
    F32, BF16 = DT.float32, DT.bfloat16
    AF = mybir.ActivationFunctionType
    ALU = mybir.AluOpType

    nc = bass.Bass()
    xT_e = nc.declare_dram_parameter("xT", [C, T], BF16, isOutput=False)
    wq_e = nc.declare_dram_parameter("wq", [C, HS], BF16, isOutput=False)
    wqs_e = nc.declare_dram_parameter("wqs", [C, HS], BF16, isOutput=False)
    wk_e = nc.declare_dram_parameter("wk", [C, HS], BF16, isOutput=False)
    wks_e = nc.declare_dram_parameter("wks", [C, HS], BF16, isOutput=False)
    wv_e = nc.declare_dram_parameter("wv", [C, HS], BF16, isOutput=False)
    cos2_e = nc.declare_dram_parameter("cos2", [HS, T], F32, isOutput=False)
    sin2s_e = nc.declare_dram_parameter("sin2s", [HS, T], F32, isOutput=False)
    maskrow_e = nc.declare_dram_parameter("maskrow", [1, T], BF16, isOutput=False)
    out_e = nc.declare_dram_parameter("out", [T, HS], F32, isOutput=True)

    with tile.TileContext(nc) as tc:
        with (
            tc.tile_pool(name="const", bufs=1) as cpool,
            tc.tile_pool(name="work", bufs=3) as wpool,
            tc.tile_pool(name="ps", bufs=2, space="PSUM") as ps,
        ):
            # ---- constants / inputs in SBUF ----
            xT = cpool.tile([C, T], BF16)
            for ch in range(NCH):
                nc.sync.dma_start(
                    out=xT[:, ch * 512 : (ch + 1) * 512],
                    in_=xT_e[:, ch * 512 : (ch + 1) * 512],
                )
            w_sb = cpool.tile([C, 5, HS], BF16)
            for i, we in enumerate((wq_e, wqs_e, wk_e, wks_e, wv_e)):
                nc.sync.dma_start(out=w_sb[:, i, :], in_=we[:, :])
            cos2 = cpool.tile([HS, T], F32)
            sin2s = cpool.tile([HS, T], F32)
            for ch in range(NCH):
                sl = slice(ch * 512, (ch + 1) * 512)
                nc.sync.dma_start(out=cos2[:, sl], in_=cos2_e[:, sl])
                nc.sync.dma_start(out=sin2s[:, sl], in_=sin2s_e[:, sl])

            identity = cpool.tile([128, 128], F32)
            masks.make_identity(nc, identity[:, :])

            # q-plus / k-plus: rows 0..63 = rope(q/k)^T, row 64 = ones / maskbias
            qplus = cpool.tile([HS + 1, T], BF16)
            kplus = cpool.tile([HS + 1, T], BF16)
            nc.gpsimd.memset(qplus[HS : HS + 1, :], 1.0)
            nc.sync.dma_start(out=kplus[HS : HS + 1, :], in_=maskrow_e[:, :])

            # v tiles + ones column: [t_in_tile, j_tile, d(65)]
            vplus = cpool.tile([128, NT, HS + 1], BF16)
            nc.gpsimd.memset(vplus[:, :, HS : HS + 1], 1.0)

            out_stage = cpool.tile([128, NT, HS], F32)

            # ---- projections + rope ----
            for ch in range(NCH):
                sl = slice(ch * 512, (ch + 1) * 512)
                q_ps = ps.tile([HS, 512], F32, tag="sgrp")
                nc.tensor.matmul(q_ps[:, :], w_sb[:, 0, :], xT[:, sl], start=True, stop=True)
                qs_ps = ps.tile([HS, 512], F32, tag="sgrp")
                nc.tensor.matmul(qs_ps[:, :], w_sb[:, 1, :], xT[:, sl], start=True, stop=True)
                m1 = wpool.tile([HS, 512], BF16, tag="rope")
                nc.vector.tensor_mul(m1[:, :], q_ps[:, :], cos2[:, sl])
                m2 = wpool.tile([HS, 512], BF16, tag="rope")
                nc.vector.tensor_mul(m2[:, :], qs_ps[:, :], sin2s[:, sl])
                nc.vector.tensor_add(qplus[0:HS, sl], m1[:, :], m2[:, :])

                k_ps = ps.tile([HS, 512], F32, tag="sgrp")
                nc.tensor.matmul(k_ps[:, :], w_sb[:, 2, :], xT[:, sl], start=True, stop=True)
                ks_ps = ps.tile([HS, 512], F32, tag="sgrp")
                nc.tensor.matmul(ks_ps[:, :], w_sb[:, 3, :], xT[:, sl], start=True, stop=True)
                m3 = wpool.tile([HS, 512], BF16, tag="rope")
                nc.vector.tensor_mul(m3[:, :], k_ps[:, :], cos2[:, sl])
                m4 = wpool.tile([HS, 512], BF16, tag="rope")
                nc.vector.tensor_mul(m4[:, :], ks_ps[:, :], sin2s[:, sl])
                nc.vector.tensor_add(kplus[0:HS, sl], m3[:, :], m4[:, :])

                for tt in range(4):
                    jt = ch * 4 + tt
                    v_ps = ps.tile([128, HS], F32, tag="sgrp")
                    nc.tensor.matmul(
                        v_ps[:, :],
                        xT[:, jt * 128 : (jt + 1) * 128],
                        w_sb[:, 4, :],
                        start=True,
                        stop=True,
                    )
                    nc.vector.tensor_copy(vplus[:, jt, 0:HS], v_ps[:, :])

            # ---- main attention loop ----
            for ic in range(NCH):
                isl = slice(ic * 512, (ic + 1) * 512)
                njt = 4 * (ic + 1)  # causal: j-tiles 0..njt-1
                outT_ps = ps.tile([HS + 1, 512], F32, tag="outT", bufs=1)
                for g0 in range(0, njt, JGRP):
                    jts = list(range(g0, min(g0 + JGRP, njt)))
                    sg_ps = ps.tile([128, JGRP * 512], F32, tag="sgrp")
                    for idx, jt in enumerate(jts):
                        nc.tensor.matmul(
                            sg_ps[:, idx * 512 : (idx + 1) * 512],
                            kplus[:, jt * 128 : (jt + 1) * 128],
                            qplus[:, isl],
                            start=True,
                            stop=True,
                        )
                    pt = wpool.tile([128, JGRP * 512], BF16, tag="pt")
                    nw = len(jts) * 512
                    nc.scalar.activation(
                        pt[:, 0:nw], sg_ps[:, 0:nw], AF.Exp, bias=0.0, scale=SCALE
                    )
                    for idx, jt in enumerate(jts):
                        psl = slice(idx * 512, idx * 512 + 512)
                        tt = jt - 4 * ic
                        if tt >= 0:
                            # diagonal-band j-tile: keep where i >= j, i.e.
                            # col - p - 128*tt >= 0 (col in i-chunk, p = j%128)
                            nc.gpsimd.affine_select(
                                out=pt[:, psl],
                                in_=pt[:, psl],
                                compare_op=ALU.is_ge,
                                fill=0.0,
                                base=-128 * tt,
                                pattern=[[1, 512]],
                                channel_multiplier=-1,
                            )
                        nc.tensor.matmul(
                            outT_ps[:, :],
                            vplus[:, jt, :],
                            pt[:, psl],
                            start=(jt == 0),
                            stop=(jt == njt - 1),
                        )

                # epilogue: transpose, normalize, stage
                outT_sb = wpool.tile([HS + 1, 512], F32, tag="outTsb", bufs=2)
                nc.vector.tensor_copy(outT_sb[:, :], outT_ps[:, :])
                for tt in range(4):
                    jt = ic * 4 + tt
                    tr_ps = ps.tile([128, HS + 1], F32, tag="tr", bufs=1)
                    nc.tensor.matmul(
                        tr_ps[:, :],
                        outT_sb[:, tt * 128 : (tt + 1) * 128],
                        identity[0 : HS + 1, 0 : HS + 1],
                        is_transpose=True,
                        start=True,
                        stop=True,
                    )
                    recip = wpool.tile([128, 1], F32, tag="recip", bufs=2)
                    nc.vector.reciprocal(recip[:, :], tr_ps[:, HS : HS + 1])
                    nc.vector.tensor_scalar_mul(
                        out_stage[:, jt, :], tr_ps[:, 0:HS], recip[:, :]
                    )

            nc.sync.dma_start(
                out=out_e.rearrange("(a p) d -> p a d", p=128),
                in_=out_stage[:, :, :],
            )

    _split_excess_waits(nc, mybir, limit=1)
    return nc


def _get_nc():
    if "nc" not in _CACHE:
        _CACHE["nc"] = _build_nc()
    return _CACHE["nc"]


def kernel(x_text_emb, Wq, Wk, Wv, freqs_cos, freqs_sin, x_latex_mask):
    import ml_dtypes
    from concourse.bass_utils import run_bass_kernel_spmd

    bf16 = ml_dtypes.bfloat16
    nc = _get_nc()

    swap = np.arange(HS) ^ 1
    cos2 = np.repeat(np.asarray(freqs_cos, np.float32).T, 2, axis=0)
    sin2s = np.repeat(np.asarray(freqs_sin, np.float32).T, 2, axis=0)
    sin2s[0::2] *= -1.0
    cos2 = np.ascontiguousarray(cos2)
    sin2s = np.ascontiguousarray(sin2s)
    wq = np.asarray(Wq, np.float32).astype(bf16)
    wqs = np.asarray(Wq, np.float32)[:, swap].astype(bf16)
    wk = np.asarray(Wk, np.float32).astype(bf16)
    wks = np.asarray(Wk, np.float32)[:, swap].astype(bf16)
    wv = np.asarray(Wv, np.float32).astype(bf16)
    maskrow = np.where(np.asarray(x_latex_mask) != 0, 0.0, NEG).astype(bf16)

    in_maps = []
    for b in range(N_CORES):
        in_maps.append(
            {
                "xT": np.ascontiguousarray(np.asarray(x_text_emb[b], np.float32).T).astype(bf16),
                "wq": wq,
                "wqs": wqs,
                "wk": wk,
                "wks": wks,
                "wv": wv,
                "cos2": cos2,
                "sin2s": sin2s,
                "maskrow": np.ascontiguousarray(maskrow[b][None, :]),
            }
        )

    res = run_bass_kernel_spmd(nc, in_maps, core_ids=list(range(N_CORES)))
    out = np.stack([res.results[b]["out"] for b in range(N_CORES)], axis=0)
    return np.asarray(out, np.float32)


# revision 12
# speedup vs baseline: 1.0318x; 1.0318x over previous
"""Single-head causal attention with RoPE + padding mask, data-parallel
over batch across 8 TRN2 NeuronCores (one batch element per core).

Per core (T=4096, C=128, HS=64):
  q = rope(x @ Wq); k = rope(x @ Wk); v = x @ Wv
  S^T[j,i] = k[j]·q[i]           (scores, transposed layout: partition=j)
  P^T = exp(S^T/sqrt(C) + maskbias[j]) * tri(i>=j)   (no max-subtraction:
        scores are O(0.1) for this problem so exp is numerically safe)
  outT[d,i] = sum_j v[j,d] P^T[j,i];  rowsum via a ones-column in v
  out[i,d] = outT[d,i] / rowsum[i]

Layout tricks (all host-side prep is pure layout/precision prep):
  - x is passed pre-transposed per-core as xT [C,T] bf16.
  - RoPE pair-swap folded into extra weight matrices Wq_swap/Wk_swap
    (swap adjacent columns), so rope = qraw*cos2 + qswap*sin2s with
    cos2/sin2s passed pre-expanded [HS,T] from host.
  - The padding mask enters as a 65th contraction row of the S^T matmul
    (k-plus row = 0/-1e30, q-plus row = 1), so exp needs no per-tile bias.
"""

import numpy as np

T, C, HS = 4096, 128, 64
N_CORES = 8
NT = T // 128      # 32 j-tiles of 128
NCH = T // 512     # 8 i-chunks of 512
JGRP = 2           # j-tiles per exp group (PSUM-bank budget bound)
SCALE = float(1.0 / np.sqrt(np.float32(C)))
NEG = -1e30

_CACHE = {}


def _install_tile_drain_patch(tile_mod):
    """This container's walrus rejects instructions with >2 sem waits; split
    Tile's final global drain into one drain per ticked processor."""
    import bass_rust
    from concourse.vector_clock import ScopedClock

    def _patched(self, tick_clock, wait_clock):
        gc = tick_clock.global_clock
        for i in range(len(gc)):
            if gc[i] <= 0:
                continue
            v = bass_rust.VectorClock()
            v.require_at_least(i, gc[i])
            d = self.nc.sync.drain()
            wait_clock.add_sem_waits(d.ins, ScopedClock({None: v}))
        self.nc.all_engine_barrier()
        assert self.sems is not None
        popped = self.nc._tile_sem_poison_stack.pop()
        assert popped is self._sem_poison
        self.nc.clear_and_free_semaphores(list(self.sems.allocated().values()))
        self.nc.all_engine_barrier()

    tile_mod.TileContext._drain_and_barrier = _patched


def _split_excess_waits(nc, mybir, limit=1):
    """This container's walrus rejects instructions with >limit sem waits.
    Hoist excess waits onto standalone EventSemaphore instructions inserted
    just before the offending instruction on the same engine queue."""
    ctr = 0
    for f in nc.m.functions:
        for b in f.blocks:
            il = b.instructions
            out = []
            changed = False
            for ins in il:
                si = ins.sync_info
                waits = list(si.on_wait) if si and si.on_wait else []
                if len(waits) > limit:
                    changed = True
                    excess = waits[: len(waits) - limit]
                    keep = waits[len(waits) - limit :]
                    for i in range(0, len(excess), limit):
                        chunk = excess[i : i + limit]
                        ev = mybir.InstEventSemaphore(
                            name=f"I-waitsplit-{ctr}",
                            engine=ins.engine,
                            ins=[],
                            outs=[],
                            sync_info=mybir.SyncInfo(on_wait=chunk, on_update=[]),
                        )
                        ctr += 1
                        nc.register_instruction(ev)
                        out.append(ev)
                    si.on_wait = keep
                out.append(ins)
            if changed:
                b.instructions = out


def _build_nc():
    import concourse.bass as bass
    import concourse.mybir as mybir
    from concourse import tile, masks

    _install_tile_drain_patch(tile)

    DT = mybir.dt
    F32, BF16 = DT.float32, DT.bfloat16
    AF = mybir.ActivationFunctionType
    ALU = mybir.AluOpType

    nc = bass.Bass()
    xT_e = nc.declare_dram_parameter("xT", [C, T], BF16, isOutput=False)
    wq_e = nc.declare_dram_parameter("wq", [C, HS], BF16, isOutput=False)
    wqs_e = nc.declare_dram_parameter("wqs", [C, HS], BF16, isOutput=False)
    wk_e = nc.declare_dram_parameter("wk", [C, HS], BF16, isOutput=False)
    wks_e = nc.declare_dram_parameter("wks", [C, HS], BF16, isOutput=False)
    wv_e = nc.declare_dram_parameter("wv", [C, HS], BF16, isOutput=False)
    cos2_e = nc.declare_dram_parameter("cos2", [HS, T], F32, isOutput=False)
    sin2s_e = nc.declare_dram_parameter("sin2s", [HS, T], F32, isOutput=False)
    mask01_e = nc.declare_dram_parameter("mask01", [128, NT], F32, isOutput=False)
    out_e = nc.declare_dram_parameter("out", [T, HS], F32, isOutput=True)

    with tile.TileContext(nc) as tc:
        with (
            tc.tile_pool(name="const", bufs=1) as cpool,
            tc.tile_pool(name="work", bufs=3) as wpool,
            tc.tile_pool(name="ps", bufs=2, space="PSUM") as ps,
        ):
            # ---- constants / inputs in SBUF ----
            xT = cpool.tile([C, T], BF16)
            for ch in range(NCH):
                nc.sync.dma_start(
                    out=xT[:, ch * 512 : (ch + 1) * 512],
                    in_=xT_e[:, ch * 512 : (ch + 1) * 512],
                )
            w_sb = cpool.tile([C, 5, HS], BF16)
            for i, we in enumerate((wq_e, wqs_e, wk_e, wks_e, wv_e)):
                nc.sync.dma_start(out=w_sb[:, i, :], in_=we[:, :])
            cos2 = cpool.tile([HS, T], F32)
            sin2s = cpool.tile([HS, T], F32)
            for ch in range(NCH):
                sl = slice(ch * 512, (ch + 1) * 512)
                nc.sync.dma_start(out=cos2[:, sl], in_=cos2_e[:, sl])
                nc.sync.dma_start(out=sin2s[:, sl], in_=sin2s_e[:, sl])

            identity = cpool.tile([128, 128], F32)
            masks.make_identity(nc, identity[:, :])

            mask01 = cpool.tile([128, NT], F32)
            nc.sync.dma_start(out=mask01[:, :], in_=mask01_e[:, :])

            # q2/k2: rows 0..63 = rope(q/k)^T, rows 64..127 duplicate for
            # row-packed (tile_position) S matmuls
            q2 = cpool.tile([128, T], BF16)
            k2 = cpool.tile([128, T], BF16)

            # v tiles + mask column (mask-weighted rowsum): [t, j_tile, d(65)]
            # padding mask applied to v rows + rowsum column instead of scores:
            # identical softmax result, keeps the S matmul at K=64.
            vplus = cpool.tile([128, NT, HS + 1], BF16)
            nc.vector.tensor_copy(vplus[:, :, HS], mask01[:, :])

            out_stage = cpool.tile([128, NT, HS], F32)

            # ---- projections + rope ----
            for ch in range(NCH):
                sl = slice(ch * 512, (ch + 1) * 512)
                q_ps = ps.tile([HS, 512], F32, tag="sgrp", bufs=3)
                nc.tensor.matmul(q_ps[:, :], w_sb[:, 0, :], xT[:, sl], start=True, stop=True)
                qs_ps = ps.tile([HS, 512], F32, tag="sgrp", bufs=3)
                nc.tensor.matmul(qs_ps[:, :], w_sb[:, 1, :], xT[:, sl], start=True, stop=True)
                m1 = wpool.tile([HS, 512], BF16, tag="rope")
                nc.vector.tensor_mul(m1[:, :], q_ps[:, :], cos2[:, sl])
                m2 = wpool.tile([HS, 512], BF16, tag="rope")
                nc.vector.tensor_mul(m2[:, :], qs_ps[:, :], sin2s[:, sl])
                nc.vector.tensor_add(q2[0:HS, sl], m1[:, :], m2[:, :])
                nc.sync.dma_start(out=q2[64:128, sl], in_=q2[0:64, sl])

                k_ps = ps.tile([HS, 512], F32, tag="sgrp", bufs=3)
                nc.tensor.matmul(k_ps[:, :], w_sb[:, 2, :], xT[:, sl], start=True, stop=True)
                ks_ps = ps.tile([HS, 512], F32, tag="sgrp", bufs=3)
                nc.tensor.matmul(ks_ps[:, :], w_sb[:, 3, :], xT[:, sl], start=True, stop=True)
                m3 = wpool.tile([HS, 512], BF16, tag="rope")
                nc.vector.tensor_mul(m3[:, :], k_ps[:, :], cos2[:, sl])
                m4 = wpool.tile([HS, 512], BF16, tag="rope")
                nc.vector.tensor_mul(m4[:, :], ks_ps[:, :], sin2s[:, sl])
                nc.vector.tensor_add(k2[0:HS, sl], m3[:, :], m4[:, :])
                nc.sync.dma_start(out=k2[64:128, sl], in_=k2[0:64, sl])

                for tt in range(4):
                    jt = ch * 4 + tt
                    v_ps = ps.tile([128, HS], F32, tag="sgrp", bufs=3)
                    nc.tensor.matmul(
                        v_ps[:, :],
                        xT[:, jt * 128 : (jt + 1) * 128],
                        w_sb[:, 4, :],
                        start=True,
                        stop=True,
                    )
                    nc.vector.tensor_scalar_mul(
                        vplus[:, jt, 0:HS], v_ps[:, :], mask01[:, jt : jt + 1]
                    )

            # ---- main attention loop ----
            for ic in range(NCH):
                isl = slice(ic * 512, (ic + 1) * 512)
                njt = 4 * (ic + 1)  # causal: j-tiles 0..njt-1
                outT_ps = ps.tile([HS + 1, 512], F32, tag="outT", bufs=1)
                for g0 in range(0, njt, JGRP):
                    jts = list(range(g0, min(g0 + JGRP, njt)))
                    sg_ps = ps.tile([128, JGRP * 512], F32, tag="sgrp", bufs=3)
                    for idx, jt in enumerate(jts):
                        # row-packed pairs: idx 0 uses PE rows 0-63, idx 1 uses
                        # rows 64-127 (duplicated q/k) — the two matmuls run
                        # concurrently on the array
                        ro = 64 * (idx % 2)
                        nc.tensor.matmul(
                            sg_ps[:, idx * 512 : (idx + 1) * 512],
                            k2[ro : ro + HS, jt * 128 : (jt + 1) * 128],
                            q2[ro : ro + HS, isl],
                            start=True,
                            stop=True,
                            tile_position=(ro, 0),
                        )
                    pt = wpool.tile([128, JGRP * 512], BF16, tag="pt")
                    nw = len(jts) * 512
                    nc.scalar.activation(
                        pt[:, 0:nw], sg_ps[:, 0:nw], AF.Exp, bias=0.0, scale=SCALE
                    )
                    for idx, jt in enumerate(jts):
                        psl = slice(idx * 512, idx * 512 + 512)
                        tt = jt - 4 * ic
                        if tt >= 0:
                            # diagonal-band j-tile: keep where i >= j, i.e.
                            # col - p - 128*tt >= 0 (col in i-chunk, p = j%128)
                            nc.gpsimd.affine_select(
                                out=pt[:, psl],
                                in_=pt[:, psl],
                                compare_op=ALU.is_ge,
                                fill=0.0,
                                base=-128 * tt,
                                pattern=[[1, 512]],
                                channel_multiplier=-1,
                            )
                        nc.tensor.matmul(
                            outT_ps[:, :],
                            vplus[:, jt, :],
                            pt[:, psl],
                            start=(jt == 0),
                            stop=(jt == njt - 1),
                        )

                # epilogue: transpose, normalize, stage
                outT_sb = wpool.tile([HS + 1, 512], F32, tag="outTsb", bufs=2)
                nc.vector.tensor_copy(outT_sb[:, :], outT_ps[:, :])
                for tt in range(4):
                    jt = ic * 4 + tt
                    tr_ps = ps.tile([128, HS + 1], F32, tag="tr", bufs=1)
                    nc.tensor.matmul(
                        tr_ps[:, :],
                        outT_sb[:, tt * 128 : (tt + 1) * 128],
                        identity[0 : HS + 1, 0 : HS + 1],
                        is_transpose=True,
                        start=True,
                        stop=True,
                    )
                    recip = wpool.tile([128, 1], F32, tag="recip", bufs=2)
                    nc.vector.reciprocal(recip[:, :], tr_ps[:, HS : HS + 1])
                    nc.vector.tensor_scalar_mul(
                        out_stage[:, jt, :], tr_ps[:, 0:HS], recip[:, :]
                    )

            nc.sync.dma_start(
                out=out_e.rearrange("(a p) d -> p a d", p=128),
                in_=out_stage[:, :, :],
            )

    _split_excess_waits(nc, mybir, limit=1)
    return nc


def _get_nc():
    if "nc" not in _CACHE:
        _CACHE["nc"] = _build_nc()
    return _CACHE["nc"]


def kernel(x_text_emb, Wq, Wk, Wv, freqs_cos, freqs_sin, x_latex_mask):
    import ml_dtypes
    from concourse.bass_utils import run_bass_kernel_spmd

    bf16 = ml_dtypes.bfloat16
    nc = _get_nc()

    swap = np.arange(HS) ^ 1
    cos2 = np.repeat(np.asarray(freqs_cos, np.float32).T, 2, axis=0)
    sin2s = np.repeat(np.asarray(freqs_sin, np.float32).T, 2, axis=0)
    sin2s[0::2] *= -1.0
    cos2 = np.ascontiguousarray(cos2)
    sin2s = np.ascontiguousarray(sin2s)
    wq = np.asarray(Wq, np.float32).astype(bf16)
    wqs = np.asarray(Wq, np.float32)[:, swap].astype(bf16)
    wk = np.asarray(Wk, np.float32).astype(bf16)
    wks = np.asarray(Wk, np.float32)[:, swap].astype(bf16)
    wv = np.asarray(Wv, np.float32).astype(bf16)
    # mask01[b] laid out [j_in_tile(128), j_tile(NT)]
    mask01 = np.asarray(x_latex_mask != 0, np.float32).reshape(N_CORES, NT, 128)

    in_maps = []
    for b in range(N_CORES):
        in_maps.append(
            {
                "xT": np.ascontiguousarray(np.asarray(x_text_emb[b], np.float32).T).astype(bf16),
                "wq": wq,
                "wqs": wqs,
                "wk": wk,
                "wks": wks,
                "wv": wv,
                "cos2": cos2,
                "sin2s": sin2s,
                "mask01": np.ascontiguousarray(mask01[b].T),
            }
        )

    res = run_bass_kernel_spmd(nc, in_maps, core_ids=list(range(N_CORES)))
    out = np.stack([res.results[b]["out"] for b in range(N_CORES)], axis=0)
    return np.asarray(out, np.float32)
